# revision 7
# baseline (speedup 1.0000x reference)
"""Chamfer distance kernel for Trainium2, 8 NeuronCores.

Math: dist2[m, n] = |y_m|^2 + |x_n|^2 - 2 y_m.x_n, computed as ONE K=24
matmul per tile using a bf16 3-way split of every operand (cross terms with
i+j<=2 kept), accumulated in fp32 PSUM -> ~1e-5 relative accuracy.
min(sqrt(d)) == sqrt(min(d)), so all mins run on squared distances and the
sqrt happens on the host over just B*(M+N) values.

Sharding: core c handles batch b = c//2, y-half h = c%2 (2048 of 4096 y
rows), all 4096 x rows.  16 m-blocks of [128 y, 4096 x] each.

v2 dataflow: the PSUM->SBUF exit (65536 f32/lane/core) is the bottleneck:
ScalarE moves 1 elem/cycle @1.2GHz and the DVE 1 elem/cycle @0.96GHz from
PSUM, so the exit is split between both engines:
  - 13 "scalar blocks": ScalarE cast-copies both [128,2048] PSUM tiles to
    one SBUF fp16 ct tile (values pre-scaled x256 so fp16 stays normal).
  - 3 "DVE blocks" (spread out): DVE extracts each PSUM half with a plain
    tensor_tensor min(pt, BIG-broadcast) -> ct half (1 elem/cycle from
    PSUM, the same speed as any PSUM read on the DVE).
Every block then gets a 2x-mode DVE fold min(ct_lo, ct_hi) -> stripe
[128, 2048]; the host finishes the 2048-way row-min from the stripes.
Column mins: one 2x-mode pair TT min(ct_2p, ct_2p+1) -> 8 partial col-acc
tiles [128, 4096] fp16 (pure functions of the cts -> loop-idempotent);
the 8-partial x 128-lane x core-half reduction happens on host.
fp16 quantization (2^-11 relative) is zero-mean across the 32k independent
min values and changes the final mean by <1e-4 relative.
"""

import numpy as np
import ml_dtypes

_B, _N, _M, _D = 4, 4096, 4096, 3
_MHALF = _M // 2
_NCORES = 8
_K = 24                  # 3-way bf16 split of [ones|norm|(-2y_d)] x [norm|ones|x_d]
_SCALE = 16.0            # per side; D2 carries x256 so fp16 mins stay normal
_BIG = 60000.0           # fp16 "infinity" for min identities
_NBLK = 16               # m-blocks per core
_DVE_BLOCKS = (5, 10, 15)  # blocks whose PSUM exit runs on the DVE

_cache = {}


def _bf16_3split(v):
    """fp32 array -> 3 bf16 parts with v ~= p0 + p1 + p2 (24 mantissa bits)."""
    v = v.astype(np.float32)
    a = v.astype(ml_dtypes.bfloat16)
    r = v - a.astype(np.float32)
    b = r.astype(ml_dtypes.bfloat16)
    c = (r - b.astype(np.float32)).astype(ml_dtypes.bfloat16)
    return [a, b, c]


# product split terms (i, j) with i+j <= 2: error floor ~2^-24 per product
_PAIR_IJ = [(0, 0), (0, 1), (1, 0), (0, 2), (2, 0), (1, 1)]


def _side_matrices(xb, yb):
    """Return (ya [24, M'], xa [24, N]) bf16 for one (batch, y-half).

    sum_k ya[k, m] * xa[k, n] ~= |y_m|^2 + |x_n|^2 - 2 y_m.x_n to ~2^-24,
    using a 3-way bf16 split of every operand:
      k0-2 : ones      <-> xnorm parts      k3-5 : ynorm parts <-> ones
      per d: (-2y_d)_i <-> (x_d)_j for (i, j) in _PAIR_IJ
    """
    n = xb.shape[0]
    m = yb.shape[0]
    xb = np.ascontiguousarray(xb, np.float32)
    yb = np.ascontiguousarray(yb, np.float32)
    xnorm = np.einsum("nd,nd->n", xb, xb, dtype=np.float32, optimize=True)
    ynorm = np.einsum("md,md->m", yb, yb, dtype=np.float32, optimize=True)
    t = (-2.0 * yb).astype(np.float32)
    ones_x = np.ones(n, ml_dtypes.bfloat16)
    ones_y = np.ones(m, ml_dtypes.bfloat16)
    ya_rows, xa_rows = [], []
    for part in _bf16_3split(xnorm):
        ya_rows.append(ones_y)
        xa_rows.append(part)
    for part in _bf16_3split(ynorm):
        ya_rows.append(part)
        xa_rows.append(ones_x)
    for d in range(_D):
        ts = _bf16_3split(t[:, d])
        xs = _bf16_3split(xb[:, d])
        for i, j in _PAIR_IJ:
            ya_rows.append(ts[i])
            xa_rows.append(xs[j])
    ya = np.stack(ya_rows).astype(np.float32) * _SCALE
    xa = np.stack(xa_rows).astype(np.float32) * _SCALE
    ya = np.ascontiguousarray(ya, dtype=ml_dtypes.bfloat16)
    xa = np.ascontiguousarray(xa, dtype=ml_dtypes.bfloat16)
    assert ya.shape[0] == _K
    return ya, xa


def _split_excess_waits(nc, mybir, maxw=1):
    """This walrus build accepts only one sync-wait per instruction; hoist
    extra waits onto wait-only Drain instructions inserted just before the
    over-limit instruction on the same engine.  (A wait-only EventSemaphore
    looks cheaper but wedges the device — empirically it must carry an
    update; Drain is safe.)"""
    n_split = 0
    for f in nc.m.functions:
        for b in f.blocks:
            il = b.instructions
            idx = 0
            while idx < len(il):
                ins = il[idx]
                si = ins.sync_info
                if si is not None and len(si.on_wait) > maxw:
                    waits = list(si.on_wait)
                    keep = waits[-maxw:]
                    extra = waits[:-maxw]
                    ins.sync_info = mybir.SyncInfo(
                        on_wait=keep, on_update=list(si.on_update)
                    )
                    for j in range(0, len(extra), maxw):
                        d = mybir.InstDrain(
                            name=f"{ins.name}-wsplit{j}",
                            engine=ins.engine,
                            ins=[],
                            outs=[],
                            sync_info=mybir.SyncInfo(
                                on_wait=extra[j : j + maxw], on_update=[]
                            ),
                        )
                        il.insert(idx, d)
                        idx += 1
                    n_split += 1
                idx += 1
    return n_split


def build_bass(loop_n=1):
    """Build the single SPMD Bass module (same program on all 8 cores).

    loop_n > 1 wraps the compute body in an on-device For_i that repeats the
    (idempotent) work — used by test.py to measure the per-iteration
    hardware time without RPC noise."""
    import contextlib
    import concourse.bass as bass
    import concourse.tile as tile
    from concourse import mybir

    MIN = mybir.AluOpType.min
    f32 = mybir.dt.float32
    bf16 = mybir.dt.bfloat16
    fp16 = mybir.dt.float16

    nc = bass.Bass(trn_type="TRN2")
    ya_d = nc.dram_tensor("ya", [_K, _MHALF], bf16, kind="ExternalInput")
    xa_d = nc.dram_tensor("xa", [_K, _N], bf16, kind="ExternalInput")
    TW = 2048                       # psum tile free width (4 banks)
    # outputs: 8 colacc partials + 16 row stripes
    cpar_d = nc.dram_tensor("cpar", [128, 8 * _N], fp16, kind="ExternalOutput")
    stri_d = nc.dram_tensor("stri", [128, _NBLK * TW], fp16, kind="ExternalOutput")

    with tile.TileContext(nc) as tc:
        with (
            tc.tile_pool(name="inputs", bufs=1) as inputs,
            tc.tile_pool(name="outs", bufs=1) as outs,
            tc.tile_pool(name="cts", bufs=4) as cts,
            tc.tile_pool(name="psum", bufs=2, space="PSUM") as psum,
        ):
            yr = inputs.tile([128, _MHALF], bf16)
            xr = inputs.tile([128, _N], bf16)
            nc.sync.dma_start(out=yr[:_K, :], in_=ya_d[:, :])
            nc.sync.dma_start(out=xr[:_K, :], in_=xa_d[:, :])
            big = inputs.tile([128, 1], fp16)
            nc.vector.memset(big, _BIG)

            cpars = [
                outs.tile([128, _N], fp16, name=f"cpar{p}", tag=f"cpar{p}")
                for p in range(8)
            ]
            stris = [
                outs.tile([128, TW], fp16, name=f"stri{s}", tag=f"stri{s}")
                for s in range(_NBLK)
            ]

            loop_cm = contextlib.ExitStack()
            if loop_n > 1:
                loop_cm.enter_context(tc.For_i(0, loop_n, 1))

            ct_pair = [None, None]
            for i in range(_NBLK):
                ct = cts.tile([128, _N], fp16)
                on_dve = i in _DVE_BLOCKS
                for j in range(2):
                    pt = psum.tile([128, TW], f32)
                    for q in range(4):
                        c0 = j * TW + q * 512
                        nc.tensor.matmul(
                            pt[:, q * 512 : (q + 1) * 512],
                            lhsT=yr[:_K, i * 128 : (i + 1) * 128],
                            rhs=xr[:_K, c0 : c0 + 512],
                            start=True,
                            stop=True,
                        )
                    dst = ct[:, j * TW : (j + 1) * TW]
                    if on_dve:
                        # DVE exit: 1x-rate PSUM read, min-with-BIG == copy
                        nc.vector.tensor_tensor(
                            out=dst,
                            in0=pt[:, :],
                            in1=big.broadcast_to((128, TW)),
                            op=MIN,
                        )
                    else:
                        nc.scalar.copy(out=dst, in_=pt[:, :])
                # row-min stripe: 2x-mode fold of the two halves
                nc.vector.tensor_tensor(
                    out=stris[i][:, :],
                    in0=ct[:, 0:TW],
                    in1=ct[:, TW : 2 * TW],
                    op=MIN,
                )
                ct_pair[i % 2] = ct
                if i % 2 == 1:
                    # column-min partial for this block pair
                    nc.vector.tensor_tensor(
                        out=cpars[i // 2][:, :],
                        in0=ct_pair[0][:, :],
                        in1=ct_pair[1][:, :],
                        op=MIN,
                    )

            loop_cm.close()
            for p in range(8):
                nc.sync.dma_start(
                    out=cpar_d[:, p * _N : (p + 1) * _N], in_=cpars[p][:, :]
                )
            for s in range(_NBLK):
                nc.sync.dma_start(
                    out=stri_d[:, s * TW : (s + 1) * TW], in_=stris[s][:, :]
                )

    _split_excess_waits(nc, mybir)
    return nc


def _get_nc():
    if "nc" not in _cache:
        _cache["nc"] = build_bass()
    return _cache["nc"]


def make_in_maps(x, y):
    """Per-core input dicts: core c -> (batch c//2, y-half c%2)."""
    x = np.asarray(x, dtype=np.float32)
    y = np.asarray(y, dtype=np.float32)
    in_maps = []
    for c in range(_NCORES):
        b, h = divmod(c, 2)
        ya, xa = _side_matrices(x[b], y[b, h * _MHALF : (h + 1) * _MHALF])
        in_maps.append({"ya": ya, "xa": xa})
    return in_maps


def reduce_outputs(results):
    """Host-side gather: per-core mins -> final scalar."""
    inv = 1.0 / (_SCALE * _SCALE)
    d2_m = np.empty((_B, _M), np.float64)
    d2_n = np.full((_B, _N), np.inf, np.float64)
    for c, r in enumerate(results):
        b, h = divmod(c, 2)
        stri = np.asarray(r["stri"]).astype(np.float64)  # [128, 16*2048]
        rowmin_blk = stri.reshape(128, _NBLK, -1).min(axis=2)  # [128, block]
        # m = i*128 + p
        d2_m[b, h * _MHALF : (h + 1) * _MHALF] = rowmin_blk.T.reshape(-1) * inv
        cpar = np.asarray(r["cpar"]).astype(np.float64)  # [128, 8*4096]
        cmin = cpar.reshape(128, 8, _N).min(axis=1).min(axis=0) * inv
        np.minimum(d2_n[b], cmin, out=d2_n[b])
    mean_m = np.sqrt(np.maximum(d2_m, 0.0)).mean()
    mean_n = np.sqrt(np.maximum(d2_n, 0.0)).mean()
    return np.float32(mean_m + mean_n)


def kernel(x, y):
    import time
    from concourse.bass_utils import run_bass_kernel_spmd

    nc = _get_nc()
    in_maps = make_in_maps(x, y)
    last_err = None
    for attempt in range(3):
        try:
            res = run_bass_kernel_spmd(nc, in_maps, core_ids=list(range(_NCORES)))
            return reduce_outputs(res.results)
        except Exception as e:  # transient axon/device hiccups: retry
            last_err = e
            time.sleep(5.0 * (attempt + 1))
    raise last_err


# revision 8
# speedup vs baseline: 1.1342x; 1.1342x over previous
"""Chamfer distance kernel for Trainium2, 8 NeuronCores.

Math: dist2[m, n] = |y_m|^2 + |x_n|^2 - 2 y_m.x_n, computed as ONE K=24
matmul per tile using a bf16 3-way split of every operand (cross terms with
i+j<=2 kept), accumulated in fp32 PSUM -> ~1e-5 relative accuracy.
min(sqrt(d)) == sqrt(min(d)), so all mins run on squared distances and the
sqrt happens on the host over just B*(M+N) values.

Sharding: core c handles batch b = c//2, y-half h = c%2 (2048 of 4096 y
rows), all 4096 x rows: 16 m-blocks of [128 y, 4096 x], processed as 8
PAIRS of m-blocks.

v3 dataflow (from hardware microbenchmarks):
  - PE row tiling: K=24 < 32, so a pair of m-blocks runs CONCURRENTLY in
    PE row-groups 0 and 2 (weights + moving operand stacked at partition
    offsets 0 and 64, tile_position (0,0)/(64,0)).  Without this the
    per-matmul LDWEIGHTS serializes with its own matmul (measured
    ~376ns/MM = 48us/core, a hidden co-bottleneck of the baseline).
  - ScalarE does ALL PSUM->SBUF exits: cast-copy [128,2048] f32 -> fp16
    ct halves (measured ~1.22us each -> 39us/core; values pre-scaled x256
    so fp16 stays normal).
  - DVE does ALL mins in fp16 2x mode: per block a fold min(ct_lo, ct_hi)
    -> stripe [128,2048] (host finishes the row-min), per pair one TT
    min(ct_even, ct_odd) -> col-min partial [128,4096] (8 partials; the
    8-partial x 128-lane x core-half reduction happens on host).
    Measured DVE total ~35us/core, overlaps ScalarE near-perfectly.
fp16 quantization (2^-11 relative) is zero-mean across the 32k independent
min values and changes the final mean by <1e-4 relative.
"""

import numpy as np
import ml_dtypes

_B, _N, _M, _D = 4, 4096, 4096, 3
_MHALF = _M // 2
_NCORES = 8
_K = 24                  # 3-way bf16 split of [ones|norm|(-2y_d)] x [norm|ones|x_d]
_SCALE = 16.0            # per side; D2 carries x256 so fp16 mins stay normal
_NBLK = 16               # m-blocks per core
_NPAIR = _NBLK // 2      # row-tiled m-block pairs
_RG = 64                 # partition offset of PE row-group for the odd block

_cache = {}


def _bf16_3split(v):
    """fp32 array -> 3 bf16 parts with v ~= p0 + p1 + p2 (24 mantissa bits)."""
    v = v.astype(np.float32)
    a = v.astype(ml_dtypes.bfloat16)
    r = v - a.astype(np.float32)
    b = r.astype(ml_dtypes.bfloat16)
    c = (r - b.astype(np.float32)).astype(ml_dtypes.bfloat16)
    return [a, b, c]


# product split terms (i, j) with i+j <= 2: error floor ~2^-24 per product
_PAIR_IJ = [(0, 0), (0, 1), (1, 0), (0, 2), (2, 0), (1, 1)]


def _side_matrices(xb, yb):
    """Return (ya [24, M'], xa [24, N]) bf16 for one (batch, y-half).

    sum_k ya[k, m] * xa[k, n] ~= |y_m|^2 + |x_n|^2 - 2 y_m.x_n to ~2^-24,
    using a 3-way bf16 split of every operand:
      k0-2 : ones      <-> xnorm parts      k3-5 : ynorm parts <-> ones
      per d: (-2y_d)_i <-> (x_d)_j for (i, j) in _PAIR_IJ
    """
    n = xb.shape[0]
    m = yb.shape[0]
    xb = np.ascontiguousarray(xb, np.float32)
    yb = np.ascontiguousarray(yb, np.float32)
    xnorm = np.einsum("nd,nd->n", xb, xb, dtype=np.float32, optimize=True)
    ynorm = np.einsum("md,md->m", yb, yb, dtype=np.float32, optimize=True)
    t = (-2.0 * yb).astype(np.float32)
    ones_x = np.ones(n, ml_dtypes.bfloat16)
    ones_y = np.ones(m, ml_dtypes.bfloat16)
    ya_rows, xa_rows = [], []
    for part in _bf16_3split(xnorm):
        ya_rows.append(ones_y)
        xa_rows.append(part)
    for part in _bf16_3split(ynorm):
        ya_rows.append(part)
        xa_rows.append(ones_x)
    for d in range(_D):
        ts = _bf16_3split(t[:, d])
        xs = _bf16_3split(xb[:, d])
        for i, j in _PAIR_IJ:
            ya_rows.append(ts[i])
            xa_rows.append(xs[j])
    ya = np.stack(ya_rows).astype(np.float32) * _SCALE
    xa = np.stack(xa_rows).astype(np.float32) * _SCALE
    ya = np.ascontiguousarray(ya, dtype=ml_dtypes.bfloat16)
    xa = np.ascontiguousarray(xa, dtype=ml_dtypes.bfloat16)
    assert ya.shape[0] == _K
    return ya, xa


def _split_excess_waits(nc, mybir, maxw=1):
    """This walrus build accepts only one sync-wait per instruction; hoist
    extra waits onto wait-only Drain instructions inserted just before the
    over-limit instruction on the same engine.  (A wait-only EventSemaphore
    looks cheaper but wedges the device — empirically it must carry an
    update; Drain is safe.)"""
    n_split = 0
    for f in nc.m.functions:
        for b in f.blocks:
            il = b.instructions
            idx = 0
            while idx < len(il):
                ins = il[idx]
                si = ins.sync_info
                if si is not None and len(si.on_wait) > maxw:
                    waits = list(si.on_wait)
                    keep = waits[-maxw:]
                    extra = waits[:-maxw]
                    ins.sync_info = mybir.SyncInfo(
                        on_wait=keep, on_update=list(si.on_update)
                    )
                    for j in range(0, len(extra), maxw):
                        d = mybir.InstDrain(
                            name=f"{ins.name}-wsplit{j}",
                            engine=ins.engine,
                            ins=[],
                            outs=[],
                            sync_info=mybir.SyncInfo(
                                on_wait=extra[j : j + maxw], on_update=[]
                            ),
                        )
                        il.insert(idx, d)
                        idx += 1
                    n_split += 1
                idx += 1
    return n_split


def build_bass(loop_n=1):
    """Build the single SPMD Bass module (same program on all 8 cores).

    loop_n > 1 wraps the compute body in an on-device For_i that repeats the
    (idempotent) work — used by test.py to measure the per-iteration
    hardware time without RPC noise."""
    import contextlib
    import concourse.bass as bass
    import concourse.tile as tile
    from concourse import mybir

    MIN = mybir.AluOpType.min
    f32 = mybir.dt.float32
    bf16 = mybir.dt.bfloat16
    fp16 = mybir.dt.float16

    nc = bass.Bass(trn_type="TRN2")
    # yab: pair g's even block weights at partitions 0:24, odd at 64:88
    yab_d = nc.dram_tensor("yab", [128, _NPAIR * 128], bf16, kind="ExternalInput")
    # xab: xa replicated at partition offsets 0 and 64
    xab_d = nc.dram_tensor("xab", [128, _N], bf16, kind="ExternalInput")
    TW = 2048                       # psum tile free width (4 banks)
    cpar_d = nc.dram_tensor("cpar", [128, _NPAIR * _N], fp16, kind="ExternalOutput")
    stri_d = nc.dram_tensor("stri", [128, _NBLK * TW], fp16, kind="ExternalOutput")

    with tile.TileContext(nc) as tc:
        with (
            tc.tile_pool(name="inputs", bufs=1) as inputs,
            tc.tile_pool(name="outs", bufs=1) as outs,
            tc.tile_pool(name="cts", bufs=4) as cts,
            tc.tile_pool(name="psum", bufs=1, space="PSUM") as psum,
        ):
            yr = inputs.tile([128, _NPAIR * 128], bf16)
            xr = inputs.tile([128, _N], bf16)
            nc.sync.dma_start(out=yr[:, :], in_=yab_d[:, :])
            nc.sync.dma_start(out=xr[:, :], in_=xab_d[:, :])

            cpars = [
                outs.tile([128, _N], fp16, name=f"cpar{p}", tag=f"cpar{p}")
                for p in range(_NPAIR)
            ]
            stris = [
                outs.tile([128, TW], fp16, name=f"stri{s}", tag=f"stri{s}")
                for s in range(_NBLK)
            ]

            loop_cm = contextlib.ExitStack()
            if loop_n > 1:
                loop_cm.enter_context(tc.For_i(0, loop_n, 1))

            for g in range(_NPAIR):
                ctA = cts.tile([128, _N], fp16, name="ctA", tag="ctA")
                ctB = cts.tile([128, _N], fp16, name="ctB", tag="ctB")
                wcol = slice(g * 128, (g + 1) * 128)
                for j in range(2):
                    ptA = psum.tile([128, TW], f32, name="ptA", tag="ptA")
                    ptB = psum.tile([128, TW], f32, name="ptB", tag="ptB")
                    for q in range(4):
                        c0 = j * TW + q * 512
                        nc.tensor.matmul(
                            ptA[:, q * 512 : (q + 1) * 512],
                            lhsT=yr[0:_K, wcol],
                            rhs=xr[0:_K, c0 : c0 + 512],
                            start=True,
                            stop=True,
                            tile_position=(0, 0),
                        )
                    for q in range(4):
                        c0 = j * TW + q * 512
                        nc.tensor.matmul(
                            ptB[:, q * 512 : (q + 1) * 512],
                            lhsT=yr[_RG : _RG + _K, wcol],
                            rhs=xr[_RG : _RG + _K, c0 : c0 + 512],
                            start=True,
                            stop=True,
                            tile_position=(_RG, 0),
                        )
                    nc.scalar.copy(out=ctA[:, j * TW : (j + 1) * TW], in_=ptA[:, :])
                    nc.scalar.copy(out=ctB[:, j * TW : (j + 1) * TW], in_=ptB[:, :])
                # row-min stripes: 2x-mode folds of each block's halves
                nc.vector.tensor_tensor(
                    out=stris[2 * g][:, :],
                    in0=ctA[:, 0:TW],
                    in1=ctA[:, TW : 2 * TW],
                    op=MIN,
                )
                nc.vector.tensor_tensor(
                    out=stris[2 * g + 1][:, :],
                    in0=ctB[:, 0:TW],
                    in1=ctB[:, TW : 2 * TW],
                    op=MIN,
                )
                # column-min partial for this pair
                nc.vector.tensor_tensor(
                    out=cpars[g][:, :],
                    in0=ctA[:, :],
                    in1=ctB[:, :],
                    op=MIN,
                )

            loop_cm.close()
            for p in range(_NPAIR):
                nc.sync.dma_start(
                    out=cpar_d[:, p * _N : (p + 1) * _N], in_=cpars[p][:, :]
                )
            for s in range(_NBLK):
                nc.sync.dma_start(
                    out=stri_d[:, s * TW : (s + 1) * TW], in_=stris[s][:, :]
                )

    _split_excess_waits(nc, mybir)
    return nc


def _get_nc():
    if "nc" not in _cache:
        _cache["nc"] = build_bass()
    return _cache["nc"]


def make_in_maps(x, y):
    """Per-core input dicts: core c -> (batch c//2, y-half c%2).

    yab stacks each m-block pair's [24, 128] weight slabs at partition
    offsets 0 (even block) and 64 (odd block); xab replicates xa at both
    offsets so each PE row-group sees its own copy of the moving operand.
    """
    x = np.asarray(x, dtype=np.float32)
    y = np.asarray(y, dtype=np.float32)
    in_maps = []
    for c in range(_NCORES):
        b, h = divmod(c, 2)
        ya, xa = _side_matrices(x[b], y[b, h * _MHALF : (h + 1) * _MHALF])
        yab = np.zeros((128, _NPAIR * 128), ml_dtypes.bfloat16)
        for g in range(_NPAIR):
            yab[0:_K, g * 128 : (g + 1) * 128] = ya[:, (2 * g) * 128 : (2 * g + 1) * 128]
            yab[_RG : _RG + _K, g * 128 : (g + 1) * 128] = ya[
                :, (2 * g + 1) * 128 : (2 * g + 2) * 128
            ]
        xab = np.zeros((128, _N), ml_dtypes.bfloat16)
        xab[0:_K] = xa
        xab[_RG : _RG + _K] = xa
        in_maps.append({"yab": yab, "xab": xab})
    return in_maps


def reduce_outputs(results):
    """Host-side gather: per-core mins -> final scalar."""
    inv = 1.0 / (_SCALE * _SCALE)
    d2_m = np.empty((_B, _M), np.float64)
    d2_n = np.full((_B, _N), np.inf, np.float64)
    for c, r in enumerate(results):
        b, h = divmod(c, 2)
        stri = np.asarray(r["stri"]).astype(np.float64)  # [128, 16*2048]
        rowmin_blk = stri.reshape(128, _NBLK, -1).min(axis=2)  # [128, block]
        # m = i*128 + p
        d2_m[b, h * _MHALF : (h + 1) * _MHALF] = rowmin_blk.T.reshape(-1) * inv
        cpar = np.asarray(r["cpar"]).astype(np.float64)  # [128, 8*4096]
        cmin = cpar.reshape(128, _NPAIR, _N).min(axis=1).min(axis=0) * inv
        np.minimum(d2_n[b], cmin, out=d2_n[b])
    mean_m = np.sqrt(np.maximum(d2_m, 0.0)).mean()
    mean_n = np.sqrt(np.maximum(d2_n, 0.0)).mean()
    return np.float32(mean_m + mean_n)


def kernel(x, y):
    import time
    from concourse.bass_utils import run_bass_kernel_spmd

    nc = _get_nc()
    in_maps = make_in_maps(x, y)
    last_err = None
    for attempt in range(3):
        try:
            res = run_bass_kernel_spmd(nc, in_maps, core_ids=list(range(_NCORES)))
            return reduce_outputs(res.results)
        except Exception as e:  # transient axon/device hiccups: retry
            last_err = e
            time.sleep(5.0 * (attempt + 1))
    raise last_err
